# revision 26
# baseline (speedup 1.0000x reference)
"""Linformer self-attention Trainium2 kernel.

Problem (hardcoded): B=4, N=4096, DIM=1024, H=16, K=256, HD=64, fp32.
  qkv = x @ Wqkv.T; q,k,v split into 16 heads of 64
  k_proj = E @ k, v_proj = F @ v  (per head, contract over tokens)
  out = softmax(q @ k_proj.T / 8) @ v_proj
  y = out @ Wout.T + bout

Sharding: 8 cores = (batch b = c//2) x (head-group hg = c%2, 8 heads each).
Each core computes a (4096, 1024) partial of y for its batch (its 512 head
dims through Wout columns); host sums hg=0 + hg=1 partials and adds bout.
No collectives.

Precision strategy (the softmax here is ~argmax: scores std ~64, so the
q/k path needs >=tf32-grade operands while the v path tolerates bf16):
  - q/k path matmuls (x->q, x->k, E@k, q@k_projT) run in float32r
    (fp32 bits, 1 cycle/row PE mode for free dim >=256; ~1.5e-4/dot HW err)
  - v path matmuls (x->v stays f32r since it shares operands; F@v, probs
    transpose, attn@v_proj, out@Wout) run in bf16
  - PSUM accumulation is fp32 everywhere; softmax exp/denoms in fp32,
    probabilities stored bf16 post-softmax.

Per-core layouts: xT (d on partitions); qT spilled to DRAM fp32 (head-dim
on partitions); k_projT/v_projT accumulated in PSUM over all 32 token
chunks (two heads share each 2KB psum zero region -> shared start/stop
flags); scores in [tok, kk] (free-axis softmax; exp fused with denom via
ACT accum_out; normalization on ACT); attnT via TensorE transpose in bf16;
h_outT kept in SBUF (bf16, no DRAM spill); final projection interleaved
per token group with phase 2.
"""

import numpy as np

B, N, DIM, H, K = 4, 4096, 1024, 16, 256
HD = DIM // H
SCALE = 1.0 / 8.0
HG = H // 2          # 8 heads per core
HGD = HG * HD        # 512 head dims per core
NCORES = 8
TG = 512             # token group
NTC = N // 128       # 32 token chunks
NGRP = N // TG       # 8 token groups

_cache = {}


def _build():
    import concourse.mybir as mybir
    import concourse.tile as tile
    from concourse import bacc
    from concourse.masks import make_identity

    f32 = mybir.dt.float32
    f32r = mybir.dt.float32r
    bf16 = mybir.dt.bfloat16
    AX = mybir.AxisListType.X
    MAX = mybir.AluOpType.max
    EXP = mybir.ActivationFunctionType.Exp

    nc = bacc.Bacc("TRN2", target_bir_lowering=False, debug=False,
                   enable_asserts=False)

    xT = nc.dram_tensor("xT", (DIM, N), f32r, kind="ExternalInput").ap()
    wqT = nc.dram_tensor("wqT", (DIM, HGD), f32r, kind="ExternalInput").ap()
    wkvT = nc.dram_tensor("wkvT", (DIM, 2 * HGD), f32r, kind="ExternalInput").ap()
    eT = nc.dram_tensor("eT", (N, HG * K), f32r, kind="ExternalInput").ap()
    fT = nc.dram_tensor("fT", (N, HG * K), bf16, kind="ExternalInput").ap()
    woutT = nc.dram_tensor("woutT", (HGD, DIM), bf16, kind="ExternalInput").ap()
    y = nc.dram_tensor("y", (N, DIM), f32, kind="ExternalOutput").ap()

    def hloc(h):
        # head h (0..7) -> (bank idx, free base) in the packed [128, 512]
        # projT psum banks: kproj on partitions 0:64 (fp32r matmul dst must
        # start at partition 0), vproj on partitions 64:128 (bf16 dst may
        # be offset). Two heads per bank, side by side in the free dim.
        return h // 2, (h % 2) * 256

    st = {}

    def phase1(tc):
        w1p, xgp, efp, kvp, qsbp = (st["w1p"], st["xgp"], st["efp"],
                                    st["kvp"], st["qsbp"])
        psq, pskv, pspr = st["psq"], st["pskv"], st["pspr"]
        projp, qT_dram = st["projp"], st["qT_dram"]

        wq_s = w1p.tile([128, 8 * HGD], f32r)
        wkv_s = w1p.tile([128, 8 * 2 * HGD], f32r)
        for dc in range(8):
            nc.sync.dma_start(wq_s[:, dc * HGD:(dc + 1) * HGD],
                              wqT[dc * 128:(dc + 1) * 128, :])
            nc.sync.dma_start(wkv_s[:, dc * 2 * HGD:(dc + 1) * 2 * HGD],
                              wkvT[dc * 128:(dc + 1) * 128, :])

        # bank j: kproj heads (2j, 2j+1) on partitions 0:64,
        #         vproj heads (2j, 2j+1) on partitions 64:128
        kvproj_ps = [pspr.tile([128, 512], f32, name=f"kvprojTps{j}")
                     for j in range(4)]

        last_gt = NTC - 1
        for g in range(NGRP):
            xg = xgp.tile([128, 8 * TG], f32r)
            for dc in range(8):
                nc.sync.dma_start(
                    xg[:, dc * TG:(dc + 1) * TG],
                    xT[dc * 128:(dc + 1) * 128, g * TG:(g + 1) * TG])
            for t in range(TG // 128):
                gt = g * (TG // 128) + t
                # q projection: qT block t over this group's tokens
                pq = psq.tile([128, 512], f32)
                for dc in range(8):
                    nc.tensor.matmul(
                        pq[:],
                        wq_s[:, dc * HGD + t * 128: dc * HGD + (t + 1) * 128],
                        xg[:, dc * TG:(dc + 1) * TG],
                        start=(dc == 0), stop=(dc == 7))
                qsb = qsbp.tile([128, 512], f32r)
                nc.scalar.copy(qsb[:], pq[:])
                nc.sync.dma_start(
                    qT_dram[t * 128:(t + 1) * 128, g * TG:(g + 1) * TG],
                    qsb[:])
                # k, v for token chunk t of this group
                eg = efp.tile([128, HG * K], f32r, name="eg")
                fg = efp.tile([128, HG * K], bf16, name="fg")
                nc.sync.dma_start(eg[:], eT[gt * 128:(gt + 1) * 128, :])
                nc.sync.dma_start(fg[:], fT[gt * 128:(gt + 1) * 128, :])
                pk = pskv.tile([128, 512], f32)
                pv = pskv.tile([128, 512], f32)
                for dc in range(8):
                    xc = xg[:, dc * TG + t * 128: dc * TG + (t + 1) * 128]
                    nc.tensor.matmul(pk[:], xc,
                                     wkv_s[:, dc * 1024: dc * 1024 + 512],
                                     start=(dc == 0), stop=(dc == 7))
                for dc in range(8):
                    xc = xg[:, dc * TG + t * 128: dc * TG + (t + 1) * 128]
                    nc.tensor.matmul(pv[:], xc,
                                     wkv_s[:, dc * 1024 + 512: dc * 1024 + 1024],
                                     start=(dc == 0), stop=(dc == 7))
                kt = kvp.tile([128, 512], f32r, name="kt")
                vt = kvp.tile([128, 512], bf16, name="vt")
                nc.vector.tensor_copy(kt[:], pk[:])
                nc.vector.tensor_copy(vt[:], pv[:])
                # accumulate k_projT / v_projT over token chunks.
                # psum zero regions are 2KB per partition row: the two heads
                # sharing a bank's partition half share one accumulation
                # group -> start on fb==0 head, stop on fb==256 head.
                for h in range(HG):
                    j, fb = hloc(h)
                    nc.tensor.matmul(
                        kvproj_ps[j][0:64, fb:fb + 256],
                        kt[:, h * 64:(h + 1) * 64],
                        eg[:, h * K:(h + 1) * K],
                        start=(gt == 0 and fb == 0),
                        stop=(gt == last_gt and fb == 256),
                        skip_group_check=True)
                for h in range(HG):
                    j, fb = hloc(h)
                    nc.tensor.matmul(
                        kvproj_ps[j][64:128, fb:fb + 256],
                        vt[:, h * 64:(h + 1) * 64],
                        fg[:, h * K:(h + 1) * K],
                        start=(gt == 0 and fb == 0),
                        stop=(gt == last_gt and fb == 256),
                        skip_group_check=True)

        # move projections to SBUF while phase-1 pools still open.
        # kpbd[hp]: block-diagonal pair layout for head pair (2hp, 2hp+1):
        #   rows 0:64   = head 2hp   k_projT at cols 0:256, zeros elsewhere
        #   rows 64:128 = head 2hp+1 k_projT at cols 256:512
        # so scores for both heads = one F=512 matmul with the stacked
        # q-pair [128 hd, tok] as stationary.
        kprojT_sb = [projp.tile([64, 512], f32r, name=f"kprojT{j}")
                     for j in range(4)]
        vprojT_sb = [projp.tile([64, 512], bf16, name=f"vprojT{j}")
                     for j in range(4)]
        kpbd = [projp.tile([128, 512], f32r, name=f"kpbd{j}")
                for j in range(4)]
        zsrc = projp.tile([128, 256], f32, name="zsrc")
        nc.vector.memset(zsrc[:], 0.0)
        for j in range(4):
            nc.vector.tensor_copy(kprojT_sb[j][:], kvproj_ps[j][0:64, :])
            nc.vector.tensor_copy(vprojT_sb[j][:], kvproj_ps[j][64:128, :])
        for j in range(4):
            # no f32r memset in the ISA: zero-fill via f32->f32r copies
            nc.vector.tensor_copy(kpbd[j][0:64, 256:512], zsrc[0:64, :])
            nc.vector.tensor_copy(kpbd[j][64:128, 0:256], zsrc[64:128, :])
            nc.vector.tensor_copy(kpbd[j][0:64, 0:256],
                                  kvproj_ps[j][0:64, 0:256])
            # partition-shifted move must go through DMA
            nc.sync.dma_start(kpbd[j][64:128, 256:512],
                              kprojT_sb[j][0:64, 256:512])
        st["kpbd"] = kpbd
        st["vprojT_sb"] = vprojT_sb

    def phase15(tc):
        ident, projp = st["ident"], st["projp"]
        vprojT_sb, psvp = st["vprojT_sb"], st["psvp"]
        vproj_sb = [projp.tile([128, 512], bf16, name=f"vproj{i}")
                    for i in range(2)]
        pvp = [psvp.tile([128, 512], bf16, name=f"pvp{kc}") for kc in range(2)]
        # vprojT_sb[j] holds heads 2j (cols 0:256) and 2j+1 (cols 256:512),
        # [64 hd, 256 K] each. Transpose each [64, 128] K-chunk into
        # vproj_sb[kc] = [128 K-chunk kc, 8 heads x 64 hd].
        for h in range(HG):
            j, fb = hloc(h)
            for kc in range(2):
                nc.tensor.transpose(
                    pvp[kc][:, h * 64:(h + 1) * 64],
                    vprojT_sb[j][0:64, fb + kc * 128: fb + (kc + 1) * 128],
                    ident[0:64, 0:64])
        for kc in range(2):
            nc.vector.tensor_copy(vproj_sb[kc][:], pvp[kc][:])
        st["vproj_sb"] = vproj_sb

    def phase2(tc):
        """Software-pipelined attention tail + interleaved output projection.

        Slot s = (g, hp) covers head pair (2hp, 2hp+1) of token group g.
        Per slot, stage A (scores + softmax) is emitted at slot s while
        stage B (transpose + AV) of slot s-1 is emitted after it, so the
        tensor queue always has independent work between dependent ops.
        Scores for both heads of a pair are one F=512 matmul against the
        block-diagonal kpbd tile. q is pre-scaled by 1/8 on the host, so
        the softmax bias is just -rowmax (negate flag on the reduce).
        """
        ident = st["ident"]
        kpbd, vproj_sb = st["kpbd"], st["vproj_sb"]
        qT_dram, hout_sb = st["qT_dram"], st["hout_sb"]
        qgp, pexpp, atp, vecp = st["qgp"], st["pexpp"], st["atp"], st["vecp"]
        pss, psat, psho = st["pss"], st["psat"], st["psho"]
        psy, wout_s, ysbp = st["psy"], st["wout_s"], st["ysbp"]

        slots = [(g, hp) for g in range(NGRP) for hp in range(4)]
        qg_t = {}

        def fetch_q(s):
            g, hp = slots[s]
            qg = qgp.tile([128, TG], f32r)
            nc.sync.dma_start(
                qg[:], qT_dram[hp * 128:(hp + 1) * 128,
                               g * TG:(g + 1) * TG])
            qg_t[s] = qg

        def stage_a(s):
            g, hp = slots[s]
            qg = qg_t.pop(s)
            pes = [[None, None] for _ in range(4)]
            for t in range(4):
                ps2 = pss.tile([128, 512], f32)
                nc.tensor.matmul(ps2[:], qg[:, t * 128:(t + 1) * 128],
                                 kpbd[hp][:], start=True, stop=True)
                for hs in range(2):
                    sl = ps2[:, hs * 256:(hs + 1) * 256]
                    negm = vecp.tile([128, 1], f32)
                    nc.vector.tensor_reduce(negm[:], sl, axis=AX, op=MAX,
                                            negate=True)
                    pe = pexpp.tile([128, 256], bf16)
                    den = vecp.tile([128, 1], f32)
                    nc.scalar.activation(pe[:], sl, EXP, bias=negm[:],
                                         scale=1.0, accum_out=den[:])
                    rec = vecp.tile([128, 1], f32)
                    nc.vector.reciprocal(rec[:], den[:])
                    nc.gpsimd.tensor_scalar_mul(pe[:], pe[:], rec[:])
                    pes[t][hs] = pe
            return pes

        def stage_b(s, pes):
            g, hp = slots[s]
            pho = psho.tile([128, 512], f32)
            for hs in range(2):
                h = hp * 2 + hs
                pat = [psat.tile([128, 512], bf16, name=f"pat{kc}",
                                 tag="pat") for kc in range(2)]
                for t in range(4):
                    for kc in range(2):
                        nc.tensor.matmul(
                            pat[kc][:, t * 128:(t + 1) * 128],
                            pes[t][hs][:, kc * 128:(kc + 1) * 128],
                            ident[:],
                            is_transpose=True, start=True, stop=True)
                at0 = atp.tile([128, 512], bf16)
                at1 = atp.tile([128, 512], bf16)
                nc.vector.tensor_copy(at0[:], pat[0][:])
                nc.vector.tensor_copy(at1[:], pat[1][:])
                for kc, at in enumerate((at0, at1)):
                    nc.tensor.matmul(
                        pho[hs * 64:(hs + 1) * 64, :],
                        vproj_sb[kc][:, h * 64:(h + 1) * 64],
                        at[:],
                        start=(kc == 0), stop=(kc == 1),
                        skip_group_check=True)
            nc.vector.tensor_copy(hout_sb[hp][:, g * TG:(g + 1) * TG], pho[:])

        def out_proj(g):
            for t in range(4):
                for ec in range(2):
                    py = psy.tile([128, 512], f32)
                    for dc in range(4):
                        nc.tensor.matmul(
                            py[:],
                            hout_sb[dc][:, (g * 4 + t) * 128:
                                        (g * 4 + t + 1) * 128],
                            wout_s[:, dc * DIM + ec * 512:
                                   dc * DIM + (ec + 1) * 512],
                            start=(dc == 0), stop=(dc == 3))
                    ysb = ysbp.tile([128, 512], f32)
                    nc.scalar.copy(ysb[:], py[:])
                    nc.sync.dma_start(
                        y[(g * 4 + t) * 128:(g * 4 + t + 1) * 128,
                          ec * 512:(ec + 1) * 512],
                        ysb[:])

        fetch_q(0)
        fetch_q(1)
        prev = None
        for s in range(len(slots)):
            if s + 2 < len(slots):
                fetch_q(s + 2)
            pes = stage_a(s)
            if prev is not None:
                stage_b(prev[0], prev[1])
                pg, php = slots[prev[0]]
                if php == 3:
                    out_proj(pg)
            prev = (s, pes)
        stage_b(prev[0], prev[1])
        out_proj(slots[prev[0]][0])

    with tile.TileContext(nc) as tc:
        with (
            tc.tile_pool(name="const", bufs=1) as constp,
            tc.tile_pool(name="proj_sb", bufs=1) as projp,
            tc.tile_pool(name="hout", bufs=1) as houtp,
            tc.tile_pool(name="dram", bufs=1, space="DRAM") as dramp,
        ):
            ident = constp.tile([128, 128], bf16)
            make_identity(nc, ident[:])
            st["ident"] = ident
            st["projp"] = projp
            st["qT_dram"] = dramp.tile([HGD, N], f32r, name="qT_dram")
            st["hout_sb"] = [houtp.tile([128, N], bf16, name=f"houts{i}")
                             for i in range(4)]

            with (
                tc.tile_pool(name="w1", bufs=1) as w1p,
                tc.tile_pool(name="xg", bufs=2) as xgp,
                tc.tile_pool(name="ef", bufs=2) as efp,
                tc.tile_pool(name="kv", bufs=3) as kvp,
                tc.tile_pool(name="qsb", bufs=2) as qsbp,
                tc.tile_pool(name="ps_q", bufs=2, space="PSUM") as psq,
                tc.tile_pool(name="ps_kv", bufs=1, space="PSUM") as pskv,
                tc.tile_pool(name="ps_proj", bufs=1, space="PSUM") as pspr,
            ):
                st.update(w1p=w1p, xgp=xgp, efp=efp, kvp=kvp, qsbp=qsbp,
                          psq=psq, pskv=pskv, pspr=pspr)
                phase1(tc)

            with tc.tile_pool(name="ps_vp", bufs=1, space="PSUM") as psvp:
                st["psvp"] = psvp
                phase15(tc)

            with tc.tile_pool(name="w3", bufs=1) as w3p:
                wout_s = w3p.tile([128, 4 * DIM], bf16)
                for dc in range(4):
                    nc.sync.dma_start(
                        wout_s[:, dc * DIM:(dc + 1) * DIM],
                        woutT[dc * 128:(dc + 1) * 128, :])
                st["wout_s"] = wout_s
                with (
                    tc.tile_pool(name="qg", bufs=4) as qgp,
                    tc.tile_pool(name="pexp", bufs=16) as pexpp,
                    tc.tile_pool(name="at", bufs=4) as atp,
                    tc.tile_pool(name="vec", bufs=64) as vecp,
                    tc.tile_pool(name="ysb", bufs=3) as ysbp,
                    tc.tile_pool(name="ps_s", bufs=3, space="PSUM") as pss,
                    tc.tile_pool(name="ps_at", bufs=2, space="PSUM") as psat,
                    tc.tile_pool(name="ps_ho", bufs=1, space="PSUM") as psho,
                    tc.tile_pool(name="ps_y", bufs=2, space="PSUM") as psy,
                ):
                    st.update(qgp=qgp, pexpp=pexpp, atp=atp, vecp=vecp,
                              ysbp=ysbp, pss=pss, psat=psat, psho=psho,
                              psy=psy)
                    phase2(tc)

    nc.compile()
    return nc


def _prep_inputs(x, Wqkv, E, F, Wout):
    """Build the 8 per-core input dicts (host-side slicing/transposes)."""
    import ml_dtypes
    bf16 = ml_dtypes.bfloat16
    ins = []
    per_hg = {}
    for hg in range(2):
        rr = hg * HGD
        # fold the 1/sqrt(HD) score scale into Wq so scores come out
        # pre-scaled and the softmax bias is just -rowmax
        wqT = np.ascontiguousarray(Wqkv[rr:rr + HGD, :].T) * SCALE
        wk = Wqkv[DIM + rr: DIM + rr + HGD, :]
        wv = Wqkv[2 * DIM + rr: 2 * DIM + rr + HGD, :]
        wkvT = np.ascontiguousarray(np.concatenate([wk.T, wv.T], axis=1))
        eT = np.ascontiguousarray(
            E[hg * HG:(hg + 1) * HG].transpose(2, 0, 1).reshape(N, HG * K))
        fT = np.ascontiguousarray(
            F[hg * HG:(hg + 1) * HG].transpose(2, 0, 1).reshape(N, HG * K)
        ).astype(bf16)
        woutT = np.ascontiguousarray(Wout[:, rr:rr + HGD].T).astype(bf16)
        per_hg[hg] = (wqT, wkvT, eT, fT, woutT)
    xTs = [np.ascontiguousarray(x[b].T) for b in range(B)]
    for c in range(NCORES):
        b, hg = c // 2, c % 2
        wqT, wkvT, eT, fT, woutT = per_hg[hg]
        ins.append({"xT": xTs[b], "wqT": wqT, "wkvT": wkvT,
                    "eT": eT, "fT": fT, "woutT": woutT})
    return ins


def kernel(x, Wqkv, E, F, Wout, bout):
    from concourse.bass_utils import run_bass_kernel_spmd

    x = np.asarray(x, dtype=np.float32)
    Wqkv = np.asarray(Wqkv, dtype=np.float32)
    E = np.asarray(E, dtype=np.float32)
    F = np.asarray(F, dtype=np.float32)
    Wout = np.asarray(Wout, dtype=np.float32)
    bout = np.asarray(bout, dtype=np.float32)

    if "nc" not in _cache:
        _cache["nc"] = _build()
    nc = _cache["nc"]

    in_maps = _prep_inputs(x, Wqkv, E, F, Wout)
    res = run_bass_kernel_spmd(nc, in_maps, core_ids=list(range(NCORES)))
    out = np.empty((B, N, DIM), dtype=np.float32)
    for b in range(B):
        out[b] = res.results[2 * b]["y"] + res.results[2 * b + 1]["y"] + bout
    return out


# revision 27
# speedup vs baseline: 1.9200x; 1.9200x over previous
"""Linformer self-attention Trainium2 kernel.

Problem (hardcoded): B=4, N=4096, DIM=1024, H=16, K=256, HD=64, fp32.
  qkv = x @ Wqkv.T; q,k,v split into 16 heads of 64
  k_proj = E @ k, v_proj = F @ v  (per head, contract over tokens)
  out = softmax(q @ k_proj.T / 8) @ v_proj
  y = out @ Wout.T + bout

Sharding: 8 cores = (batch b = c//2) x (head-group hg = c%2, 8 heads each).
Each core computes a (4096, 1024) partial of y for its batch (its 512 head
dims through Wout columns); host sums hg=0 + hg=1 partials and adds bout.
No collectives.

Precision strategy (the softmax here is ~argmax: scores std ~64, so the
q/k path needs >=tf32-grade operands while the v path tolerates bf16):
  - q/k path matmuls (x->q, x->k, E@k, q@k_projT) run in float32r
    (fp32 bits, 1 cycle/row PE mode for free dim >=256; ~1.5e-4/dot HW err)
  - v path matmuls (x->v stays f32r since it shares operands; F@v, probs
    transpose, attn@v_proj, out@Wout) run in bf16
  - PSUM accumulation is fp32 everywhere; softmax exp/denoms in fp32,
    probabilities stored bf16 post-softmax.

Per-core layouts: xT (d on partitions); qT spilled to DRAM fp32 (head-dim
on partitions); k_projT/v_projT accumulated in PSUM over all 32 token
chunks (two heads share each 2KB psum zero region -> shared start/stop
flags); scores in [tok, kk] (free-axis softmax; exp fused with denom via
ACT accum_out; normalization on ACT); attnT via TensorE transpose in bf16;
h_outT kept in SBUF (bf16, no DRAM spill); final projection interleaved
per token group with phase 2.
"""

import numpy as np

B, N, DIM, H, K = 4, 4096, 1024, 16, 256
HD = DIM // H
SCALE = 1.0 / 8.0
HG = H // 2          # 8 heads per core
HGD = HG * HD        # 512 head dims per core
NCORES = 8
TG = 512             # token group
NTC = N // 128       # 32 token chunks
NGRP = N // TG       # 8 token groups

_cache = {}


def _build():
    import concourse.mybir as mybir
    import concourse.tile as tile
    from concourse import bacc
    from concourse.masks import make_identity

    f32 = mybir.dt.float32
    f32r = mybir.dt.float32r
    bf16 = mybir.dt.bfloat16
    AX = mybir.AxisListType.X
    MAX = mybir.AluOpType.max
    EXP = mybir.ActivationFunctionType.Exp

    nc = bacc.Bacc("TRN2", target_bir_lowering=False, debug=False,
                   enable_asserts=False)

    xT = nc.dram_tensor("xT", (DIM, N), f32r, kind="ExternalInput").ap()
    wqT = nc.dram_tensor("wqT", (DIM, HGD), f32r, kind="ExternalInput").ap()
    wkvT = nc.dram_tensor("wkvT", (DIM, 2 * HGD), f32r, kind="ExternalInput").ap()
    eT = nc.dram_tensor("eT", (N, HG * K), f32r, kind="ExternalInput").ap()
    fT = nc.dram_tensor("fT", (N, HG * K), bf16, kind="ExternalInput").ap()
    woutT = nc.dram_tensor("woutT", (HGD, DIM), bf16, kind="ExternalInput").ap()
    y = nc.dram_tensor("y", (N, DIM), f32, kind="ExternalOutput").ap()

    def hloc(h):
        # head h (0..7) -> (bank idx, free base) in the packed [128, 512]
        # projT psum banks: kproj on partitions 0:64 (fp32r matmul dst must
        # start at partition 0), vproj on partitions 64:128 (bf16 dst may
        # be offset). Two heads per bank, side by side in the free dim.
        return h // 2, (h % 2) * 256

    st = {}

    def phase1(tc):
        w1p, xgp, efp, kvp, qsbp = (st["w1p"], st["xgp"], st["efp"],
                                    st["kvp"], st["qsbp"])
        psq, pskv, pspr = st["psq"], st["pskv"], st["pspr"]
        projp, qT_dram = st["projp"], st["qT_dram"]

        wq_s = w1p.tile([128, 8 * HGD], f32r)
        wkv_s = w1p.tile([128, 8 * 2 * HGD], f32r)
        for dc in range(8):
            nc.sync.dma_start(wq_s[:, dc * HGD:(dc + 1) * HGD],
                              wqT[dc * 128:(dc + 1) * 128, :])
            nc.sync.dma_start(wkv_s[:, dc * 2 * HGD:(dc + 1) * 2 * HGD],
                              wkvT[dc * 128:(dc + 1) * 128, :])

        # bank j: kproj heads (2j, 2j+1) on partitions 0:64,
        #         vproj heads (2j, 2j+1) on partitions 64:128
        kvproj_ps = [pspr.tile([128, 512], f32, name=f"kvprojTps{j}")
                     for j in range(4)]

        last_gt = NTC - 1
        for g in range(NGRP):
            xg = xgp.tile([128, 8 * TG], f32r)
            for dc in range(8):
                nc.sync.dma_start(
                    xg[:, dc * TG:(dc + 1) * TG],
                    xT[dc * 128:(dc + 1) * 128, g * TG:(g + 1) * TG])
            for t in range(TG // 128):
                gt = g * (TG // 128) + t
                # q projection: qT block t over this group's tokens
                pq = psq.tile([128, 512], f32)
                for dc in range(8):
                    nc.tensor.matmul(
                        pq[:],
                        wq_s[:, dc * HGD + t * 128: dc * HGD + (t + 1) * 128],
                        xg[:, dc * TG:(dc + 1) * TG],
                        start=(dc == 0), stop=(dc == 7))
                qsb = qsbp.tile([128, 512], f32r)
                nc.scalar.copy(qsb[:], pq[:])
                nc.sync.dma_start(
                    qT_dram[t * 128:(t + 1) * 128, g * TG:(g + 1) * TG],
                    qsb[:])
                # k, v for token chunk t of this group
                eg = efp.tile([128, HG * K], f32r, name="eg")
                fg = efp.tile([128, HG * K], bf16, name="fg")
                nc.sync.dma_start(eg[:], eT[gt * 128:(gt + 1) * 128, :])
                nc.sync.dma_start(fg[:], fT[gt * 128:(gt + 1) * 128, :])
                pk = pskv.tile([128, 512], f32)
                pv = pskv.tile([128, 512], f32)
                for dc in range(8):
                    xc = xg[:, dc * TG + t * 128: dc * TG + (t + 1) * 128]
                    nc.tensor.matmul(pk[:], xc,
                                     wkv_s[:, dc * 1024: dc * 1024 + 512],
                                     start=(dc == 0), stop=(dc == 7))
                for dc in range(8):
                    xc = xg[:, dc * TG + t * 128: dc * TG + (t + 1) * 128]
                    nc.tensor.matmul(pv[:], xc,
                                     wkv_s[:, dc * 1024 + 512: dc * 1024 + 1024],
                                     start=(dc == 0), stop=(dc == 7))
                kt = kvp.tile([128, 512], f32r, name="kt")
                vt = kvp.tile([128, 512], bf16, name="vt")
                nc.vector.tensor_copy(kt[:], pk[:])
                nc.vector.tensor_copy(vt[:], pv[:])
                # accumulate k_projT / v_projT over token chunks.
                # psum zero regions are 2KB per partition row: the two heads
                # sharing a bank's partition half share one accumulation
                # group -> start on fb==0 head, stop on fb==256 head.
                for h in range(HG):
                    j, fb = hloc(h)
                    nc.tensor.matmul(
                        kvproj_ps[j][0:64, fb:fb + 256],
                        kt[:, h * 64:(h + 1) * 64],
                        eg[:, h * K:(h + 1) * K],
                        start=(gt == 0 and fb == 0),
                        stop=(gt == last_gt and fb == 256),
                        skip_group_check=True)
                for h in range(HG):
                    j, fb = hloc(h)
                    nc.tensor.matmul(
                        kvproj_ps[j][64:128, fb:fb + 256],
                        vt[:, h * 64:(h + 1) * 64],
                        fg[:, h * K:(h + 1) * K],
                        start=(gt == 0 and fb == 0),
                        stop=(gt == last_gt and fb == 256),
                        skip_group_check=True)

        # move projections to SBUF while phase-1 pools still open.
        # kpbd[hp]: block-diagonal pair layout for head pair (2hp, 2hp+1):
        #   rows 0:64   = head 2hp   k_projT at cols 0:256, zeros elsewhere
        #   rows 64:128 = head 2hp+1 k_projT at cols 256:512
        # so scores for both heads = one F=512 matmul with the stacked
        # q-pair [128 hd, tok] as stationary.
        kprojT_sb = [projp.tile([64, 512], f32r, name=f"kprojT{j}")
                     for j in range(4)]
        vprojT_sb = [projp.tile([64, 512], bf16, name=f"vprojT{j}")
                     for j in range(4)]
        kpbd = [projp.tile([128, 512], f32r, name=f"kpbd{j}")
                for j in range(4)]
        zsrc = projp.tile([128, 256], f32, name="zsrc")
        nc.vector.memset(zsrc[:], 0.0)
        for j in range(4):
            nc.vector.tensor_copy(kprojT_sb[j][:], kvproj_ps[j][0:64, :])
            nc.vector.tensor_copy(vprojT_sb[j][:], kvproj_ps[j][64:128, :])
        for j in range(4):
            # no f32r memset in the ISA: zero-fill via f32->f32r copies
            nc.vector.tensor_copy(kpbd[j][0:64, 256:512], zsrc[0:64, :])
            nc.vector.tensor_copy(kpbd[j][64:128, 0:256], zsrc[64:128, :])
            nc.vector.tensor_copy(kpbd[j][0:64, 0:256],
                                  kvproj_ps[j][0:64, 0:256])
            # partition-shifted move must go through DMA
            nc.sync.dma_start(kpbd[j][64:128, 256:512],
                              kprojT_sb[j][0:64, 256:512])
        st["kpbd"] = kpbd
        st["vprojT_sb"] = vprojT_sb

    def phase15(tc):
        ident, projp = st["ident"], st["projp"]
        vprojT_sb, psvp = st["vprojT_sb"], st["psvp"]
        vproj_sb = [projp.tile([128, 512], bf16, name=f"vproj{i}")
                    for i in range(2)]
        pvp = [psvp.tile([128, 512], bf16, name=f"pvp{kc}") for kc in range(2)]
        # vprojT_sb[j] holds heads 2j (cols 0:256) and 2j+1 (cols 256:512),
        # [64 hd, 256 K] each. Transpose each [64, 128] K-chunk into
        # vproj_sb[kc] = [128 K-chunk kc, 8 heads x 64 hd].
        for h in range(HG):
            j, fb = hloc(h)
            for kc in range(2):
                nc.tensor.transpose(
                    pvp[kc][:, h * 64:(h + 1) * 64],
                    vprojT_sb[j][0:64, fb + kc * 128: fb + (kc + 1) * 128],
                    ident[0:64, 0:64])
        for kc in range(2):
            nc.vector.tensor_copy(vproj_sb[kc][:], pvp[kc][:])
        st["vproj_sb"] = vproj_sb

    def phase2(tc):
        """Software-pipelined attention tail + interleaved output projection.

        Slot s = (g, hp) covers head pair (2hp, 2hp+1) of token group g.
        Per slot, stage A (scores + softmax) is emitted at slot s while
        stage B (transpose + AV) of slot s-1 is emitted after it, so the
        tensor queue always has independent work between dependent ops.
        Scores for both heads of a pair are one F=512 matmul against the
        block-diagonal kpbd tile. q is pre-scaled by 1/8 on the host, so
        the softmax bias is just -rowmax (negate flag on the reduce).
        """
        ident = st["ident"]
        kpbd, vproj_sb = st["kpbd"], st["vproj_sb"]
        qT_dram, hout_sb = st["qT_dram"], st["hout_sb"]
        qgp, pexpp, atp, vecp = st["qgp"], st["pexpp"], st["atp"], st["vecp"]
        pss, psat, psho = st["pss"], st["psat"], st["psho"]
        psy, wout_s, ysbp = st["psy"], st["wout_s"], st["ysbp"]

        slots = [(g, hp) for g in range(NGRP) for hp in range(4)]
        qg_t = {}

        def fetch_q(s):
            g, hp = slots[s]
            qg = qgp.tile([128, TG], f32r)
            nc.sync.dma_start(
                qg[:], qT_dram[hp * 128:(hp + 1) * 128,
                               g * TG:(g + 1) * TG])
            qg_t[s] = qg

        def stage_a(s):
            g, hp = slots[s]
            qg = qg_t.pop(s)
            pes = [[None, None] for _ in range(4)]
            for t in range(4):
                ps2 = pss.tile([128, 512], f32)
                nc.tensor.matmul(ps2[:], qg[:, t * 128:(t + 1) * 128],
                                 kpbd[hp][:], start=True, stop=True)
                for hs in range(2):
                    sl = ps2[:, hs * 256:(hs + 1) * 256]
                    negm = vecp.tile([128, 1], f32)
                    nc.vector.tensor_reduce(negm[:], sl, axis=AX, op=MAX,
                                            negate=True)
                    pe = pexpp.tile([128, 256], bf16)
                    den = vecp.tile([128, 1], f32)
                    nc.scalar.activation(pe[:], sl, EXP, bias=negm[:],
                                         scale=1.0, accum_out=den[:])
                    rec = vecp.tile([128, 1], f32)
                    nc.vector.reciprocal(rec[:], den[:])
                    nc.vector.tensor_scalar_mul(pe[:], pe[:], rec[:])
                    pes[t][hs] = pe
            return pes

        def stage_b(s, pes):
            g, hp = slots[s]
            pho = psho.tile([128, 512], f32)
            for hs in range(2):
                h = hp * 2 + hs
                pat = [psat.tile([128, 512], bf16, name=f"pat{kc}",
                                 tag="pat") for kc in range(2)]
                for t in range(4):
                    for kc in range(2):
                        nc.tensor.matmul(
                            pat[kc][:, t * 128:(t + 1) * 128],
                            pes[t][hs][:, kc * 128:(kc + 1) * 128],
                            ident[:],
                            is_transpose=True, start=True, stop=True)
                at0 = atp.tile([128, 512], bf16)
                at1 = atp.tile([128, 512], bf16)
                nc.vector.tensor_copy(at0[:], pat[0][:])
                nc.vector.tensor_copy(at1[:], pat[1][:])
                for kc, at in enumerate((at0, at1)):
                    nc.tensor.matmul(
                        pho[hs * 64:(hs + 1) * 64, :],
                        vproj_sb[kc][:, h * 64:(h + 1) * 64],
                        at[:],
                        start=(kc == 0), stop=(kc == 1),
                        skip_group_check=True)
            nc.vector.tensor_copy(hout_sb[hp][:, g * TG:(g + 1) * TG], pho[:])

        def out_proj(g):
            for t in range(4):
                for ec in range(2):
                    py = psy.tile([128, 512], f32)
                    for dc in range(4):
                        nc.tensor.matmul(
                            py[:],
                            hout_sb[dc][:, (g * 4 + t) * 128:
                                        (g * 4 + t + 1) * 128],
                            wout_s[:, dc * DIM + ec * 512:
                                   dc * DIM + (ec + 1) * 512],
                            start=(dc == 0), stop=(dc == 3))
                    ysb = ysbp.tile([128, 512], f32)
                    nc.scalar.copy(ysb[:], py[:])
                    nc.sync.dma_start(
                        y[(g * 4 + t) * 128:(g * 4 + t + 1) * 128,
                          ec * 512:(ec + 1) * 512],
                        ysb[:])

        fetch_q(0)
        fetch_q(1)
        prev = None
        for s in range(len(slots)):
            if s + 2 < len(slots):
                fetch_q(s + 2)
            pes = stage_a(s)
            if prev is not None:
                stage_b(prev[0], prev[1])
                pg, php = slots[prev[0]]
                if php == 3:
                    out_proj(pg)
            prev = (s, pes)
        stage_b(prev[0], prev[1])
        out_proj(slots[prev[0]][0])

    with tile.TileContext(nc) as tc:
        with (
            tc.tile_pool(name="const", bufs=1) as constp,
            tc.tile_pool(name="proj_sb", bufs=1) as projp,
            tc.tile_pool(name="hout", bufs=1) as houtp,
            tc.tile_pool(name="dram", bufs=1, space="DRAM") as dramp,
        ):
            ident = constp.tile([128, 128], bf16)
            make_identity(nc, ident[:])
            st["ident"] = ident
            st["projp"] = projp
            st["qT_dram"] = dramp.tile([HGD, N], f32r, name="qT_dram")
            st["hout_sb"] = [houtp.tile([128, N], bf16, name=f"houts{i}")
                             for i in range(4)]

            with (
                tc.tile_pool(name="w1", bufs=1) as w1p,
                tc.tile_pool(name="xg", bufs=2) as xgp,
                tc.tile_pool(name="ef", bufs=2) as efp,
                tc.tile_pool(name="kv", bufs=3) as kvp,
                tc.tile_pool(name="qsb", bufs=2) as qsbp,
                tc.tile_pool(name="ps_q", bufs=2, space="PSUM") as psq,
                tc.tile_pool(name="ps_kv", bufs=1, space="PSUM") as pskv,
                tc.tile_pool(name="ps_proj", bufs=1, space="PSUM") as pspr,
            ):
                st.update(w1p=w1p, xgp=xgp, efp=efp, kvp=kvp, qsbp=qsbp,
                          psq=psq, pskv=pskv, pspr=pspr)
                phase1(tc)

            with tc.tile_pool(name="ps_vp", bufs=1, space="PSUM") as psvp:
                st["psvp"] = psvp
                phase15(tc)

            with tc.tile_pool(name="w3", bufs=1) as w3p:
                wout_s = w3p.tile([128, 4 * DIM], bf16)
                for dc in range(4):
                    nc.sync.dma_start(
                        wout_s[:, dc * DIM:(dc + 1) * DIM],
                        woutT[dc * 128:(dc + 1) * 128, :])
                st["wout_s"] = wout_s
                with (
                    tc.tile_pool(name="qg", bufs=4) as qgp,
                    tc.tile_pool(name="pexp", bufs=16) as pexpp,
                    tc.tile_pool(name="at", bufs=4) as atp,
                    tc.tile_pool(name="vec", bufs=64) as vecp,
                    tc.tile_pool(name="ysb", bufs=3) as ysbp,
                    tc.tile_pool(name="ps_s", bufs=3, space="PSUM") as pss,
                    tc.tile_pool(name="ps_at", bufs=2, space="PSUM") as psat,
                    tc.tile_pool(name="ps_ho", bufs=1, space="PSUM") as psho,
                    tc.tile_pool(name="ps_y", bufs=2, space="PSUM") as psy,
                ):
                    st.update(qgp=qgp, pexpp=pexpp, atp=atp, vecp=vecp,
                              ysbp=ysbp, pss=pss, psat=psat, psho=psho,
                              psy=psy)
                    phase2(tc)

    nc.compile()
    return nc


def _prep_inputs(x, Wqkv, E, F, Wout):
    """Build the 8 per-core input dicts (host-side slicing/transposes)."""
    import ml_dtypes
    bf16 = ml_dtypes.bfloat16
    ins = []
    per_hg = {}
    for hg in range(2):
        rr = hg * HGD
        # fold the 1/sqrt(HD) score scale into Wq so scores come out
        # pre-scaled and the softmax bias is just -rowmax
        wqT = np.ascontiguousarray(Wqkv[rr:rr + HGD, :].T) * SCALE
        wk = Wqkv[DIM + rr: DIM + rr + HGD, :]
        wv = Wqkv[2 * DIM + rr: 2 * DIM + rr + HGD, :]
        wkvT = np.ascontiguousarray(np.concatenate([wk.T, wv.T], axis=1))
        eT = np.ascontiguousarray(
            E[hg * HG:(hg + 1) * HG].transpose(2, 0, 1).reshape(N, HG * K))
        fT = np.ascontiguousarray(
            F[hg * HG:(hg + 1) * HG].transpose(2, 0, 1).reshape(N, HG * K)
        ).astype(bf16)
        woutT = np.ascontiguousarray(Wout[:, rr:rr + HGD].T).astype(bf16)
        per_hg[hg] = (wqT, wkvT, eT, fT, woutT)
    xTs = [np.ascontiguousarray(x[b].T) for b in range(B)]
    for c in range(NCORES):
        b, hg = c // 2, c % 2
        wqT, wkvT, eT, fT, woutT = per_hg[hg]
        ins.append({"xT": xTs[b], "wqT": wqT, "wkvT": wkvT,
                    "eT": eT, "fT": fT, "woutT": woutT})
    return ins


def kernel(x, Wqkv, E, F, Wout, bout):
    from concourse.bass_utils import run_bass_kernel_spmd

    x = np.asarray(x, dtype=np.float32)
    Wqkv = np.asarray(Wqkv, dtype=np.float32)
    E = np.asarray(E, dtype=np.float32)
    F = np.asarray(F, dtype=np.float32)
    Wout = np.asarray(Wout, dtype=np.float32)
    bout = np.asarray(bout, dtype=np.float32)

    if "nc" not in _cache:
        _cache["nc"] = _build()
    nc = _cache["nc"]

    in_maps = _prep_inputs(x, Wqkv, E, F, Wout)
    res = run_bass_kernel_spmd(nc, in_maps, core_ids=list(range(NCORES)))
    out = np.empty((B, N, DIM), dtype=np.float32)
    for b in range(B):
        out[b] = res.results[2 * b]["y"] + res.results[2 * b + 1]["y"] + bout
    return out


# revision 41
# speedup vs baseline: 1.9397x; 1.0102x over previous
"""Linformer self-attention Trainium2 kernel.

Problem (hardcoded): B=4, N=4096, DIM=1024, H=16, K=256, HD=64, fp32.
  qkv = x @ Wqkv.T; q,k,v split into 16 heads of 64
  k_proj = E @ k, v_proj = F @ v  (per head, contract over tokens)
  out = softmax(q @ k_proj.T / 8) @ v_proj
  y = out @ Wout.T + bout

Sharding: 8 cores = (batch b = c//2) x (head-group hg = c%2, 8 heads each).
Each core computes a (4096, 1024) partial of y for its batch (its 512 head
dims through Wout columns); host sums hg=0 + hg=1 partials and adds bout.
No collectives.

Precision strategy (the softmax here is ~argmax: scores std ~64, so the
q/k path needs >=tf32-grade operands while the v path tolerates bf16):
  - q/k path matmuls (x->q, x->k, E@k, q@k_projT) run in float32r
    (fp32 bits, 1 cycle/row PE mode for free dim >=256; ~1.5e-4/dot HW err)
  - v path matmuls (x->v stays f32r since it shares operands; F@v, probs
    transpose, attn@v_proj, out@Wout) run in bf16
  - PSUM accumulation is fp32 everywhere; softmax exp/denoms in fp32,
    probabilities stored bf16 post-softmax.

Per-core layouts: xT (d on partitions); qT spilled to DRAM fp32 (head-dim
on partitions); k_projT/v_projT accumulated in PSUM over all 32 token
chunks (two heads share each 2KB psum zero region -> shared start/stop
flags); scores in [tok, kk] (free-axis softmax; exp fused with denom via
ACT accum_out; normalization on ACT); attnT via TensorE transpose in bf16;
h_outT kept in SBUF (bf16, no DRAM spill); final projection interleaved
per token group with phase 2.
"""

import numpy as np

B, N, DIM, H, K = 4, 4096, 1024, 16, 256
HD = DIM // H
SCALE = 1.0 / 8.0
HG = H // 2          # 8 heads per core
HGD = HG * HD        # 512 head dims per core
NCORES = 8
TG = 512             # token group
NTC = N // 128       # 32 token chunks
NGRP = N // TG       # 8 token groups

_cache = {}


def _build():
    import concourse.mybir as mybir
    import concourse.tile as tile
    from concourse import bacc
    from concourse.masks import make_identity

    f32 = mybir.dt.float32
    f32r = mybir.dt.float32r
    bf16 = mybir.dt.bfloat16
    AX = mybir.AxisListType.X
    MAX = mybir.AluOpType.max
    EXP = mybir.ActivationFunctionType.Exp

    nc = bacc.Bacc("TRN2", target_bir_lowering=False, debug=False,
                   enable_asserts=False)

    xT = nc.dram_tensor("xT", (DIM, N), f32r, kind="ExternalInput").ap()
    wqT = nc.dram_tensor("wqT", (DIM, HGD), f32r, kind="ExternalInput").ap()
    wkvT = nc.dram_tensor("wkvT", (DIM, 2 * HGD), f32r, kind="ExternalInput").ap()
    eT = nc.dram_tensor("eT", (N, HG * K), f32r, kind="ExternalInput").ap()
    fT = nc.dram_tensor("fT", (N, HG * K), bf16, kind="ExternalInput").ap()
    woutT = nc.dram_tensor("woutT", (HGD, DIM), bf16, kind="ExternalInput").ap()
    y = nc.dram_tensor("y", (N, DIM), f32, kind="ExternalOutput").ap()

    def hloc(h):
        # head h (0..7) -> (bank idx, free base) in the packed [128, 512]
        # projT psum banks: kproj on partitions 0:64 (fp32r matmul dst must
        # start at partition 0), vproj on partitions 64:128 (bf16 dst may
        # be offset). Two heads per bank, side by side in the free dim.
        return h // 2, (h % 2) * 256

    st = {}

    def phase1(tc):
        w1p, xgp, efp, kvp, qsbp = (st["w1p"], st["xgp"], st["efp"],
                                    st["kvp"], st["qsbp"])
        psq, pskv, pspr = st["psq"], st["pskv"], st["pspr"]
        projp, qT_dram = st["projp"], st["qT_dram"]

        wq_s = w1p.tile([128, 8 * HGD], f32r)
        wkv_s = w1p.tile([128, 8 * 2 * HGD], f32r)
        # dc0 weights first, in small chunks spread over DMA queues, so the
        # very first matmuls aren't gated on one 256KB transfer; the rest of
        # the weights are emitted after group 0's x/E/F loads (deps keep
        # everything correct, this only shapes queue order).
        for c in range(4):
            nc.sync.dma_start(wq_s[:, c * 128:(c + 1) * 128],
                              wqT[0:128, c * 128:(c + 1) * 128])
        for c in range(4):
            nc.sync.dma_start(wkv_s[:, c * 256:(c + 1) * 256],
                              wkvT[0:128, c * 256:(c + 1) * 256])

        def load_rest_weights():
            for dc in range(1, 8):
                nc.sync.dma_start(wq_s[:, dc * HGD:(dc + 1) * HGD],
                                  wqT[dc * 128:(dc + 1) * 128, :])
                nc.sync.dma_start(
                    wkv_s[:, dc * 2 * HGD:(dc + 1) * 2 * HGD],
                    wkvT[dc * 128:(dc + 1) * 128, :])

        # bank j: kproj heads (2j, 2j+1) on partitions 0:64,
        #         vproj heads (2j, 2j+1) on partitions 64:128
        kvproj_ps = [pspr.tile([128, 512], f32, name=f"kvprojTps{j}")
                     for j in range(4)]

        last_gt = NTC - 1
        for g in range(NGRP):
            xg = xgp.tile([128, 8 * TG], f32r)
            nchunk = 2 if g == 0 else 1
            for dc in range(8):
                for c in range(nchunk):
                    w = TG // nchunk
                    nc.sync.dma_start(
                        xg[:, dc * TG + c * w: dc * TG + (c + 1) * w],
                        xT[dc * 128:(dc + 1) * 128,
                           g * TG + c * w: g * TG + (c + 1) * w])
            if g == 0:
                load_rest_weights()
            for t in range(TG // 128):
                gt = g * (TG // 128) + t
                # q projection: qT block t over this group's tokens
                pq = psq.tile([128, 512], f32)
                for dc in range(8):
                    nc.tensor.matmul(
                        pq[:],
                        wq_s[:, dc * HGD + t * 128: dc * HGD + (t + 1) * 128],
                        xg[:, dc * TG:(dc + 1) * TG],
                        start=(dc == 0), stop=(dc == 7))
                qsb = qsbp.tile([128, 512], f32r)
                nc.scalar.copy(qsb[:], pq[:])
                nc.sync.dma_start(
                    qT_dram[t * 128:(t + 1) * 128, g * TG:(g + 1) * TG],
                    qsb[:])
                # k, v for token chunk t of this group
                eg = efp.tile([128, HG * K], f32r, name="eg")
                fg = efp.tile([128, HG * K], bf16, name="fg")
                nc.sync.dma_start(eg[:], eT[gt * 128:(gt + 1) * 128, :])
                nc.sync.dma_start(fg[:], fT[gt * 128:(gt + 1) * 128, :])
                pk = pskv.tile([128, 512], f32)
                pv = pskv.tile([128, 512], f32)
                for dc in range(8):
                    xc = xg[:, dc * TG + t * 128: dc * TG + (t + 1) * 128]
                    nc.tensor.matmul(pk[:], xc,
                                     wkv_s[:, dc * 1024: dc * 1024 + 512],
                                     start=(dc == 0), stop=(dc == 7))
                for dc in range(8):
                    xc = xg[:, dc * TG + t * 128: dc * TG + (t + 1) * 128]
                    nc.tensor.matmul(pv[:], xc,
                                     wkv_s[:, dc * 1024 + 512: dc * 1024 + 1024],
                                     start=(dc == 0), stop=(dc == 7))
                kt = kvp.tile([128, 512], f32r, name="kt")
                vt = kvp.tile([128, 512], bf16, name="vt")
                nc.vector.tensor_copy(kt[:], pk[:])
                nc.vector.tensor_copy(vt[:], pv[:])
                # accumulate k_projT / v_projT over token chunks.
                # psum zero regions are 2KB per partition row: the two heads
                # sharing a bank's partition half share one accumulation
                # group -> start on fb==0 head, stop on fb==256 head.
                for h in range(HG):
                    j, fb = hloc(h)
                    nc.tensor.matmul(
                        kvproj_ps[j][0:64, fb:fb + 256],
                        kt[:, h * 64:(h + 1) * 64],
                        eg[:, h * K:(h + 1) * K],
                        start=(gt == 0 and fb == 0),
                        stop=(gt == last_gt and fb == 256),
                        skip_group_check=True)
                for h in range(HG):
                    j, fb = hloc(h)
                    nc.tensor.matmul(
                        kvproj_ps[j][64:128, fb:fb + 256],
                        vt[:, h * 64:(h + 1) * 64],
                        fg[:, h * K:(h + 1) * K],
                        start=(gt == 0 and fb == 0),
                        stop=(gt == last_gt and fb == 256),
                        skip_group_check=True)

        # move projections to SBUF while phase-1 pools still open.
        # kpbd[hp]: block-diagonal pair layout for head pair (2hp, 2hp+1):
        #   rows 0:64   = head 2hp   k_projT at cols 0:256, zeros elsewhere
        #   rows 64:128 = head 2hp+1 k_projT at cols 256:512
        # so scores for both heads = one F=512 matmul with the stacked
        # q-pair [128 hd, tok] as stationary.
        kprojT_sb = [projp.tile([64, 512], f32r, name=f"kprojT{j}")
                     for j in range(4)]
        vprojT_sb = [projp.tile([64, 512], bf16, name=f"vprojT{j}")
                     for j in range(4)]
        kpbd = [projp.tile([128, 512], f32r, name=f"kpbd{j}")
                for j in range(4)]
        zsrc = projp.tile([128, 256], f32, name="zsrc")
        nc.vector.memset(zsrc[:], 0.0)
        for j in range(4):
            nc.vector.tensor_copy(kprojT_sb[j][:], kvproj_ps[j][0:64, :])
            nc.vector.tensor_copy(vprojT_sb[j][:], kvproj_ps[j][64:128, :])
        for j in range(4):
            # no f32r memset in the ISA: zero-fill via f32->f32r copies
            nc.vector.tensor_copy(kpbd[j][0:64, 256:512], zsrc[0:64, :])
            nc.vector.tensor_copy(kpbd[j][64:128, 0:256], zsrc[64:128, :])
            nc.vector.tensor_copy(kpbd[j][0:64, 0:256],
                                  kvproj_ps[j][0:64, 0:256])
            # partition-shifted move must go through DMA
            nc.sync.dma_start(kpbd[j][64:128, 256:512],
                              kprojT_sb[j][0:64, 256:512])
        st["kpbd"] = kpbd
        st["vprojT_sb"] = vprojT_sb

    def phase15(tc):
        ident, projp = st["ident"], st["projp"]
        vprojT_sb, psvp = st["vprojT_sb"], st["psvp"]
        vproj_sb = [projp.tile([128, 512], bf16, name=f"vproj{i}")
                    for i in range(2)]
        pvp = [psvp.tile([128, 512], bf16, name=f"pvp{kc}") for kc in range(2)]
        # vprojT_sb[j] holds heads 2j (cols 0:256) and 2j+1 (cols 256:512),
        # [64 hd, 256 K] each. Transpose each [64, 128] K-chunk into
        # vproj_sb[kc] = [128 K-chunk kc, 8 heads x 64 hd].
        for h in range(HG):
            j, fb = hloc(h)
            for kc in range(2):
                nc.tensor.transpose(
                    pvp[kc][:, h * 64:(h + 1) * 64],
                    vprojT_sb[j][0:64, fb + kc * 128: fb + (kc + 1) * 128],
                    ident[0:64, 0:64])
        for kc in range(2):
            nc.vector.tensor_copy(vproj_sb[kc][:], pvp[kc][:])
        st["vproj_sb"] = vproj_sb

    def phase2(tc):
        """Software-pipelined attention tail + interleaved output projection.

        Slot s = (g, hp) covers head pair (2hp, 2hp+1) of token group g.
        Per slot, stage A (scores + softmax) is emitted at slot s while
        stage B (transpose + AV) of slot s-1 is emitted after it, so the
        tensor queue always has independent work between dependent ops.
        Scores for both heads of a pair are one F=512 matmul against the
        block-diagonal kpbd tile. q is pre-scaled by 1/8 on the host, so
        the softmax bias is just -rowmax (negate flag on the reduce).
        """
        kpbd, vproj_sb = st["kpbd"], st["vproj_sb"]
        qT_dram, hout_sb = st["qT_dram"], st["hout_sb"]
        qgp, pexpp, atp, vecp = st["qgp"], st["pexpp"], st["atp"], st["vecp"]
        pss, psb = st["pss"], st["psb"]
        wout_s, ysbp = st["wout_s"], st["ysbp"]
        ident, dtiles = st["ident"], st["dtiles"]

        slots = [(g, hp) for g in range(NGRP) for hp in range(4)]
        qg_t = {}

        def fetch_q(s):
            g, hp = slots[s]
            qg = qgp.tile([128, TG], f32r)
            nc.sync.dma_start(
                qg[:], qT_dram[hp * 128:(hp + 1) * 128,
                               g * TG:(g + 1) * TG])
            qg_t[s] = qg

        def stage_a_part(s, ts, qg, pes, ds):
            g, hp = slots[s]
            for t in ts:
                ps2 = pss.tile([128, 512], f32)
                nc.tensor.matmul(ps2[:], qg[:, t * 128:(t + 1) * 128],
                                 kpbd[hp][:], start=True, stop=True)
                for hs in range(2):
                    sl = ps2[:, hs * 256:(hs + 1) * 256]
                    negm = vecp.tile([128, 1], f32)
                    nc.vector.tensor_reduce(negm[:], sl, axis=AX, op=MAX,
                                            negate=True)
                    pe = pexpp.tile([128, 256], bf16)
                    den = vecp.tile([128, 1], f32)
                    nc.scalar.activation(pe[:], sl, EXP, bias=negm[:],
                                         scale=1.0, accum_out=den[:])
                    rec = vecp.tile([128, 1], f32)
                    nc.vector.reciprocal(rec[:], den[:])
                    # build diag(1/den) for this token chunk: identity
                    # scaled per-partition by rec. The transpose matmul
                    # pe^T @ D then yields normalized attn^T directly.
                    nc.vector.tensor_scalar_mul(
                        ds[hs][:, t * 128:(t + 1) * 128],
                        ident[:], rec[:])
                    pes[t][hs] = pe

        def stage_b(s, pes, ds):
            """Transpose+normalize (pe^T @ D), then AV, for slot s."""
            g, hp = slots[s]
            pho = psb.tile([128, 512], f32, name="pho", tag="psb")
            for hs in range(2):
                h = hp * 2 + hs
                pat = [psb.tile([128, 512], f32, name=f"pat{kc}", tag="psb")
                       for kc in range(2)]
                for t in range(4):
                    for kc in range(2):
                        nc.tensor.matmul(
                            pat[kc][:, t * 128:(t + 1) * 128],
                            pes[t][hs][:, kc * 128:(kc + 1) * 128],
                            ds[hs][:, t * 128:(t + 1) * 128],
                            start=True, stop=True)
                at0 = atp.tile([128, 512], bf16)
                at1 = atp.tile([128, 512], bf16)
                nc.vector.tensor_copy(at0[:], pat[0][:])
                nc.vector.tensor_copy(at1[:], pat[1][:])
                for kc, at in enumerate((at0, at1)):
                    nc.tensor.matmul(
                        pho[hs * 64:(hs + 1) * 64, :],
                        vproj_sb[kc][:, h * 64:(h + 1) * 64],
                        at[:],
                        start=(kc == 0), stop=(kc == 1),
                        skip_group_check=True)
            nc.vector.tensor_copy(hout_sb[hp][:, g * TG:(g + 1) * TG], pho[:])

        def out_proj(g):
            for t in range(4):
                for ec in range(2):
                    py = psb.tile([128, 512], f32, name="py", tag="psb")
                    for dc in range(4):
                        nc.tensor.matmul(
                            py[:],
                            hout_sb[dc][:, (g * 4 + t) * 128:
                                        (g * 4 + t + 1) * 128],
                            wout_s[:, dc * DIM + ec * 512:
                                   dc * DIM + (ec + 1) * 512],
                            start=(dc == 0), stop=(dc == 3))
                    ysb = ysbp.tile([128, 512], f32)
                    nc.scalar.copy(ysb[:], py[:])
                    nc.sync.dma_start(
                        y[(g * 4 + t) * 128:(g * 4 + t + 1) * 128,
                          ec * 512:(ec + 1) * 512],
                        ysb[:])

        fetch_q(0)
        fetch_q(1)
        prev = None
        for s in range(len(slots)):
            if s + 2 < len(slots):
                fetch_q(s + 2)
            qg = qg_t.pop(s)
            pes = [[None, None] for _ in range(4)]
            ds = (dtiles[(s % 2) * 2], dtiles[(s % 2) * 2 + 1])
            stage_a_part(s, (0, 1), qg, pes, ds)
            if prev is not None:
                stage_b(*prev)
                pg, php = slots[prev[0]]
                if php == 3:
                    out_proj(pg)
            stage_a_part(s, (2, 3), qg, pes, ds)
            prev = (s, pes, ds)
        stage_b(*prev)
        out_proj(slots[prev[0]][0])

    with tile.TileContext(nc) as tc:
        with (
            tc.tile_pool(name="const", bufs=1) as constp,
            tc.tile_pool(name="proj_sb", bufs=1) as projp,
            tc.tile_pool(name="hout", bufs=1) as houtp,
            tc.tile_pool(name="dram", bufs=1, space="DRAM") as dramp,
        ):
            ident = constp.tile([128, 128], bf16)
            make_identity(nc, ident[:])
            st["ident"] = ident
            st["projp"] = projp
            st["qT_dram"] = dramp.tile([HGD, N], f32r, name="qT_dram")
            st["hout_sb"] = [houtp.tile([128, N], bf16, name=f"houts{i}")
                             for i in range(4)]

            with (
                tc.tile_pool(name="w1", bufs=1) as w1p,
                tc.tile_pool(name="xg", bufs=2) as xgp,
                tc.tile_pool(name="ef", bufs=2) as efp,
                tc.tile_pool(name="kv", bufs=3) as kvp,
                tc.tile_pool(name="qsb", bufs=2) as qsbp,
                tc.tile_pool(name="ps_q", bufs=2, space="PSUM") as psq,
                tc.tile_pool(name="ps_kv", bufs=1, space="PSUM") as pskv,
                tc.tile_pool(name="ps_proj", bufs=1, space="PSUM") as pspr,
            ):
                st.update(w1p=w1p, xgp=xgp, efp=efp, kvp=kvp, qsbp=qsbp,
                          psq=psq, pskv=pskv, pspr=pspr)
                phase1(tc)

            with tc.tile_pool(name="ps_vp", bufs=1, space="PSUM") as psvp:
                st["psvp"] = psvp
                phase15(tc)

            with tc.tile_pool(name="w3", bufs=1) as w3p:
                wout_s = w3p.tile([128, 4 * DIM], bf16)
                for dc in range(4):
                    nc.sync.dma_start(
                        wout_s[:, dc * DIM:(dc + 1) * DIM],
                        woutT[dc * 128:(dc + 1) * 128, :])
                st["wout_s"] = wout_s
                with (
                    tc.tile_pool(name="qg", bufs=4) as qgp,
                    tc.tile_pool(name="pexp", bufs=16) as pexpp,
                    tc.tile_pool(name="at", bufs=4) as atp,
                    tc.tile_pool(name="vec", bufs=64) as vecp,
                    tc.tile_pool(name="ysb", bufs=3) as ysbp,
                    tc.tile_pool(name="dt", bufs=1) as dtp,
                    tc.tile_pool(name="ps_s", bufs=2, space="PSUM") as pss,
                    tc.tile_pool(name="ps_b", bufs=6, space="PSUM") as psb,
                ):
                    dtiles = [dtp.tile([128, 512], bf16, name=f"dt{i}")
                              for i in range(4)]
                    st.update(qgp=qgp, pexpp=pexpp, atp=atp, vecp=vecp,
                              ysbp=ysbp, pss=pss, psb=psb, dtiles=dtiles)
                    phase2(tc)

    nc.compile()
    return nc


def _prep_inputs(x, Wqkv, E, F, Wout):
    """Build the 8 per-core input dicts (host-side slicing/transposes)."""
    import ml_dtypes
    bf16 = ml_dtypes.bfloat16
    ins = []
    per_hg = {}
    for hg in range(2):
        rr = hg * HGD
        # fold the 1/sqrt(HD) score scale into Wq so scores come out
        # pre-scaled and the softmax bias is just -rowmax
        wqT = np.ascontiguousarray(Wqkv[rr:rr + HGD, :].T) * SCALE
        wk = Wqkv[DIM + rr: DIM + rr + HGD, :]
        wv = Wqkv[2 * DIM + rr: 2 * DIM + rr + HGD, :]
        wkvT = np.ascontiguousarray(np.concatenate([wk.T, wv.T], axis=1))
        eT = np.ascontiguousarray(
            E[hg * HG:(hg + 1) * HG].transpose(2, 0, 1).reshape(N, HG * K))
        fT = np.ascontiguousarray(
            F[hg * HG:(hg + 1) * HG].transpose(2, 0, 1).reshape(N, HG * K)
        ).astype(bf16)
        woutT = np.ascontiguousarray(Wout[:, rr:rr + HGD].T).astype(bf16)
        per_hg[hg] = (wqT, wkvT, eT, fT, woutT)
    xTs = [np.ascontiguousarray(x[b].T) for b in range(B)]
    for c in range(NCORES):
        b, hg = c // 2, c % 2
        wqT, wkvT, eT, fT, woutT = per_hg[hg]
        ins.append({"xT": xTs[b], "wqT": wqT, "wkvT": wkvT,
                    "eT": eT, "fT": fT, "woutT": woutT})
    return ins


def kernel(x, Wqkv, E, F, Wout, bout):
    from concourse.bass_utils import run_bass_kernel_spmd

    x = np.asarray(x, dtype=np.float32)
    Wqkv = np.asarray(Wqkv, dtype=np.float32)
    E = np.asarray(E, dtype=np.float32)
    F = np.asarray(F, dtype=np.float32)
    Wout = np.asarray(Wout, dtype=np.float32)
    bout = np.asarray(bout, dtype=np.float32)

    if "nc" not in _cache:
        _cache["nc"] = _build()
    nc = _cache["nc"]

    in_maps = _prep_inputs(x, Wqkv, E, F, Wout)
    res = run_bass_kernel_spmd(nc, in_maps, core_ids=list(range(NCORES)))
    out = np.empty((B, N, DIM), dtype=np.float32)
    for b in range(B):
        out[b] = res.results[2 * b]["y"] + res.results[2 * b + 1]["y"] + bout
    return out


# revision 54
# speedup vs baseline: 2.0760x; 1.0703x over previous
"""Linformer self-attention Trainium2 kernel.

Problem (hardcoded): B=4, N=4096, DIM=1024, H=16, K=256, HD=64, fp32.
  qkv = x @ Wqkv.T; q,k,v split into 16 heads of 64
  k_proj = E @ k, v_proj = F @ v  (per head, contract over tokens)
  out = softmax(q @ k_proj.T / 8) @ v_proj
  y = out @ Wout.T + bout

Sharding: 8 cores = (batch b = c//2) x (head-group hg = c%2, 8 heads each).
Each core computes a (4096, 1024) partial of y for its batch (its 512 head
dims through Wout columns); host sums hg=0 + hg=1 partials and adds bout.
No collectives.

Precision strategy (the softmax here is ~argmax: scores std ~64, so the
q/k path needs >=tf32-grade operands while the v path tolerates bf16):
  - q/k path matmuls (x->q, x->k, E@k, q@k_projT) run in float32r
    (fp32 bits, 1 cycle/row PE mode for free dim >=256; ~1.5e-4/dot HW err)
  - v path matmuls (x->v stays f32r since it shares operands; F@v, probs
    transpose, attn@v_proj, out@Wout) run in bf16
  - PSUM accumulation is fp32 everywhere; softmax exp/denoms in fp32,
    probabilities stored bf16 post-softmax.

Per-core layouts: xT (d on partitions); qT spilled to DRAM fp32 (head-dim
on partitions); k_projT/v_projT accumulated in PSUM over all 32 token
chunks (two heads share each 2KB psum zero region -> shared start/stop
flags); scores in [tok, kk] (free-axis softmax; exp fused with denom via
ACT accum_out; normalization on ACT); attnT via TensorE transpose in bf16;
h_outT kept in SBUF (bf16, no DRAM spill); final projection interleaved
per token group with phase 2.
"""

import numpy as np

B, N, DIM, H, K = 4, 4096, 1024, 16, 256
HD = DIM // H
SCALE = 1.0 / 8.0
HG = H // 2          # 8 heads per core
HGD = HG * HD        # 512 head dims per core
NCORES = 8
TG = 512             # token group
NTC = N // 128       # 32 token chunks
NGRP = N // TG       # 8 token groups

_cache = {}


def _build():
    import concourse.mybir as mybir
    import concourse.tile as tile
    from concourse import bacc
    from concourse.masks import make_identity

    f32 = mybir.dt.float32
    f32r = mybir.dt.float32r
    bf16 = mybir.dt.bfloat16
    AX = mybir.AxisListType.X
    MAX = mybir.AluOpType.max
    EXP = mybir.ActivationFunctionType.Exp

    nc = bacc.Bacc("TRN2", target_bir_lowering=False, debug=False,
                   enable_asserts=False)

    xT = nc.dram_tensor("xT", (DIM, N), f32r, kind="ExternalInput").ap()
    wqT = nc.dram_tensor("wqT", (DIM, HGD), f32r, kind="ExternalInput").ap()
    wkvT = nc.dram_tensor("wkvT", (DIM, 2 * HGD), f32r, kind="ExternalInput").ap()
    eT = nc.dram_tensor("eT", (N, HG * K), f32r, kind="ExternalInput").ap()
    fT = nc.dram_tensor("fT", (N, HG * K), bf16, kind="ExternalInput").ap()
    woutT = nc.dram_tensor("woutT", (HGD, DIM), bf16, kind="ExternalInput").ap()
    # y partials are summed (and upcast) on the host; bf16 halves the
    # output staging copies and DMA
    y = nc.dram_tensor("y", (N, DIM), bf16, kind="ExternalOutput").ap()

    def hloc(h):
        # head h (0..7) -> (bank idx, free base) in the packed [128, 512]
        # projT psum banks: kproj on partitions 0:64 (fp32r matmul dst must
        # start at partition 0), vproj on partitions 64:128 (bf16 dst may
        # be offset). Two heads per bank, side by side in the free dim.
        return h // 2, (h % 2) * 256

    st = {}

    def phase1(tc):
        w1p, xgp, efp, kvp, qsbp = (st["w1p"], st["xgp"], st["efp"],
                                    st["kvp"], st["qsbp"])
        psq, pskv, pspr = st["psq"], st["pskv"], st["pspr"]
        projp, qT_dram = st["projp"], st["qT_dram"]

        wq_s = w1p.tile([128, 8 * HGD], f32r)
        wkv_s = w1p.tile([128, 8 * 2 * HGD], f32r)
        # dc0 weights first, in small chunks spread over DMA queues, so the
        # very first matmuls aren't gated on one 256KB transfer; the rest of
        # the weights are emitted after group 0's x/E/F loads (deps keep
        # everything correct, this only shapes queue order).
        for c in range(4):
            nc.sync.dma_start(wq_s[:, c * 128:(c + 1) * 128],
                              wqT[0:128, c * 128:(c + 1) * 128])
        for c in range(4):
            nc.sync.dma_start(wkv_s[:, c * 256:(c + 1) * 256],
                              wkvT[0:128, c * 256:(c + 1) * 256])

        def load_rest_weights():
            for dc in range(1, 8):
                nc.sync.dma_start(wq_s[:, dc * HGD:(dc + 1) * HGD],
                                  wqT[dc * 128:(dc + 1) * 128, :])
                nc.sync.dma_start(
                    wkv_s[:, dc * 2 * HGD:(dc + 1) * 2 * HGD],
                    wkvT[dc * 128:(dc + 1) * 128, :])

        # bank j: kproj heads (2j, 2j+1) on partitions 0:64,
        #         vproj heads (2j, 2j+1) on partitions 64:128
        kvproj_ps = [pspr.tile([128, 512], f32, name=f"kvprojTps{j}")
                     for j in range(4)]

        last_gt = NTC - 1
        for g in range(NGRP):
            xg = xgp.tile([128, 8 * TG], f32r)
            nchunk = 2 if g == 0 else 1
            for dc in range(8):
                for c in range(nchunk):
                    w = TG // nchunk
                    nc.sync.dma_start(
                        xg[:, dc * TG + c * w: dc * TG + (c + 1) * w],
                        xT[dc * 128:(dc + 1) * 128,
                           g * TG + c * w: g * TG + (c + 1) * w])
            if g == 0:
                load_rest_weights()
            for t in range(TG // 128):
                gt = g * (TG // 128) + t
                # q projection: qT block t over this group's tokens
                pq = psq.tile([128, 512], f32)
                for dc in range(8):
                    nc.tensor.matmul(
                        pq[:],
                        wq_s[:, dc * HGD + t * 128: dc * HGD + (t + 1) * 128],
                        xg[:, dc * TG:(dc + 1) * TG],
                        start=(dc == 0), stop=(dc == 7))
                qsb = qsbp.tile([128, 512], f32r)
                nc.scalar.copy(qsb[:], pq[:])
                nc.sync.dma_start(
                    qT_dram[t * 128:(t + 1) * 128, g * TG:(g + 1) * TG],
                    qsb[:])
                # k, v for token chunk t of this group
                eg = efp.tile([128, HG * K], f32r, name="eg")
                fg = efp.tile([128, HG * K], bf16, name="fg")
                nc.sync.dma_start(eg[:], eT[gt * 128:(gt + 1) * 128, :])
                nc.sync.dma_start(fg[:], fT[gt * 128:(gt + 1) * 128, :])
                pk = pskv.tile([128, 512], f32)
                pv = pskv.tile([128, 512], f32)
                for dc in range(8):
                    xc = xg[:, dc * TG + t * 128: dc * TG + (t + 1) * 128]
                    nc.tensor.matmul(pk[:], xc,
                                     wkv_s[:, dc * 1024: dc * 1024 + 512],
                                     start=(dc == 0), stop=(dc == 7))
                for dc in range(8):
                    xc = xg[:, dc * TG + t * 128: dc * TG + (t + 1) * 128]
                    nc.tensor.matmul(pv[:], xc,
                                     wkv_s[:, dc * 1024 + 512: dc * 1024 + 1024],
                                     start=(dc == 0), stop=(dc == 7))
                kt = kvp.tile([128, 512], f32r, name="kt")
                vt = kvp.tile([128, 512], bf16, name="vt")
                nc.vector.tensor_copy(kt[:], pk[:])
                nc.vector.tensor_copy(vt[:], pv[:])
                # accumulate k_projT / v_projT over token chunks.
                # psum zero regions are 2KB per partition row: the two heads
                # sharing a bank's partition half share one accumulation
                # group -> start on fb==0 head, stop on fb==256 head.
                for h in range(HG):
                    j, fb = hloc(h)
                    nc.tensor.matmul(
                        kvproj_ps[j][0:64, fb:fb + 256],
                        kt[:, h * 64:(h + 1) * 64],
                        eg[:, h * K:(h + 1) * K],
                        start=(gt == 0 and fb == 0),
                        stop=(gt == last_gt and fb == 256),
                        skip_group_check=True)
                for h in range(HG):
                    j, fb = hloc(h)
                    nc.tensor.matmul(
                        kvproj_ps[j][64:128, fb:fb + 256],
                        vt[:, h * 64:(h + 1) * 64],
                        fg[:, h * K:(h + 1) * K],
                        start=(gt == 0 and fb == 0),
                        stop=(gt == last_gt and fb == 256),
                        skip_group_check=True)

        # move projections to SBUF while phase-1 pools still open.
        # kpbd[hp]: block-diagonal pair layout for head pair (2hp, 2hp+1):
        #   rows 0:64   = head 2hp   k_projT at cols 0:256, zeros elsewhere
        #   rows 64:128 = head 2hp+1 k_projT at cols 256:512
        # so scores for both heads = one F=512 matmul with the stacked
        # q-pair [128 hd, tok] as stationary.
        kprojT_sb = [projp.tile([64, 512], f32r, name=f"kprojT{j}")
                     for j in range(4)]
        vprojT_sb = [projp.tile([64, 512], bf16, name=f"vprojT{j}")
                     for j in range(4)]
        kpbd = [projp.tile([128, 512], f32r, name=f"kpbd{j}")
                for j in range(4)]
        zsrc = projp.tile([128, 256], f32, name="zsrc")
        nc.vector.memset(zsrc[:], 0.0)
        for j in range(4):
            nc.vector.tensor_copy(kprojT_sb[j][:], kvproj_ps[j][0:64, :])
            nc.vector.tensor_copy(vprojT_sb[j][:], kvproj_ps[j][64:128, :])
        for j in range(4):
            # no f32r memset in the ISA: zero-fill via f32->f32r copies
            nc.vector.tensor_copy(kpbd[j][0:64, 256:512], zsrc[0:64, :])
            nc.vector.tensor_copy(kpbd[j][64:128, 0:256], zsrc[64:128, :])
            nc.vector.tensor_copy(kpbd[j][0:64, 0:256],
                                  kvproj_ps[j][0:64, 0:256])
            # partition-shifted move must go through DMA
            nc.sync.dma_start(kpbd[j][64:128, 256:512],
                              kprojT_sb[j][0:64, 256:512])
        st["kpbd"] = kpbd
        st["vprojT_sb"] = vprojT_sb

    def phase15(tc):
        ident, projp = st["ident"], st["projp"]
        vprojT_sb, psvp = st["vprojT_sb"], st["psvp"]
        vproj_sb = [projp.tile([128, 512], bf16, name=f"vproj{i}")
                    for i in range(2)]
        pvp = [psvp.tile([128, 512], bf16, name=f"pvp{kc}") for kc in range(2)]
        # vprojT_sb[j] holds heads 2j (cols 0:256) and 2j+1 (cols 256:512),
        # [64 hd, 256 K] each. Transpose each [64, 128] K-chunk into
        # vproj_sb[kc] = [128 K-chunk kc, 8 heads x 64 hd].
        for h in range(HG):
            j, fb = hloc(h)
            for kc in range(2):
                nc.tensor.transpose(
                    pvp[kc][:, h * 64:(h + 1) * 64],
                    vprojT_sb[j][0:64, fb + kc * 128: fb + (kc + 1) * 128],
                    ident[0:64, 0:64])
        for kc in range(2):
            nc.vector.tensor_copy(vproj_sb[kc][:], pvp[kc][:])
        st["vproj_sb"] = vproj_sb

    def phase2(tc):
        """Software-pipelined attention tail + interleaved output projection.

        Slot s = (g, hp) covers head pair (2hp, 2hp+1) of token group g.
        Per slot, stage A (scores + softmax) is emitted at slot s while
        stage B (transpose + AV) of slot s-1 is emitted after it, so the
        tensor queue always has independent work between dependent ops.
        Scores for both heads of a pair are one F=512 matmul against the
        block-diagonal kpbd tile. q is pre-scaled by 1/8 on the host, so
        the softmax bias is just -rowmax (negate flag on the reduce).
        """
        kpbd, vproj_sb = st["kpbd"], st["vproj_sb"]
        qT_dram, hout_sb = st["qT_dram"], st["hout_sb"]
        qgp, pexpp, atp, vecp = st["qgp"], st["pexpp"], st["atp"], st["vecp"]
        pss, psb = st["pss"], st["psb"]
        wout_s, ysbp = st["wout_s"], st["ysbp"]
        ident, dtiles = st["ident"], st["dtiles"]

        slots = [(g, hp) for g in range(NGRP) for hp in range(4)]
        qg_t = {}

        def fetch_q(s):
            g, hp = slots[s]
            qg = qgp.tile([128, TG], f32r)
            nc.sync.dma_start(
                qg[:], qT_dram[hp * 128:(hp + 1) * 128,
                               g * TG:(g + 1) * TG])
            qg_t[s] = qg

        def stage_a_part(s, ts, qg, pes, ds):
            g, hp = slots[s]
            for t in ts:
                ps2 = pss.tile([128, 512], f32)
                nc.tensor.matmul(ps2[:], qg[:, t * 128:(t + 1) * 128],
                                 kpbd[hp][:], start=True, stop=True)
                for hs in range(2):
                    sl = ps2[:, hs * 256:(hs + 1) * 256]
                    negm = vecp.tile([128, 1], f32)
                    nc.vector.tensor_reduce(negm[:], sl, axis=AX, op=MAX,
                                            negate=True)
                    pe = pexpp.tile([128, 256], bf16)
                    den = vecp.tile([128, 1], f32)
                    nc.scalar.activation(pe[:], sl, EXP, bias=negm[:],
                                         scale=1.0, accum_out=den[:])
                    rec = vecp.tile([128, 1], f32)
                    nc.vector.reciprocal(rec[:], den[:])
                    # build diag(1/den) for this token chunk: identity
                    # scaled per-partition by 1/den. The transpose matmul
                    # pe^T @ D then yields normalized attn^T directly.
                    nc.vector.tensor_scalar_mul(
                        ds[hs][:, t * 128:(t + 1) * 128],
                        ident[:], rec[:])
                    pes[t][hs] = pe

        def stage_b(s, pes, ds):
            """Transpose+normalize (pe^T @ D), then AV, for slot s."""
            g, hp = slots[s]
            pho = psb.tile([128, 512], f32, name="pho", tag="hy", bufs=2)
            for hs in range(2):
                h = hp * 2 + hs
                pat = [psb.tile([128, 512], f32, name=f"pat{kc}", tag="pat",
                                bufs=3) for kc in range(2)]
                for t in range(4):
                    for kc in range(2):
                        nc.tensor.matmul(
                            pat[kc][:, t * 128:(t + 1) * 128],
                            pes[t][hs][:, kc * 128:(kc + 1) * 128],
                            ds[hs][:, t * 128:(t + 1) * 128],
                            start=True, stop=True)
                at0 = atp.tile([128, 512], bf16)
                at1 = atp.tile([128, 512], bf16)
                nc.vector.tensor_copy(at0[:], pat[0][:])
                nc.vector.tensor_copy(at1[:], pat[1][:])
                for kc, at in enumerate((at0, at1)):
                    nc.tensor.matmul(
                        pho[hs * 64:(hs + 1) * 64, :],
                        vproj_sb[kc][:, h * 64:(h + 1) * 64],
                        at[:],
                        start=(kc == 0), stop=(kc == 1),
                        skip_group_check=True)
            nc.scalar.copy(hout_sb[hp][:, g * TG:(g + 1) * TG], pho[:])

        def out_proj(g):
            for t in range(4):
                for ec in range(2):
                    py = psb.tile([128, 512], f32, name="py", tag="hy",
                                  bufs=2)
                    for dc in range(4):
                        nc.tensor.matmul(
                            py[:],
                            hout_sb[dc][:, (g * 4 + t) * 128:
                                        (g * 4 + t + 1) * 128],
                            wout_s[:, dc * DIM + ec * 512:
                                   dc * DIM + (ec + 1) * 512],
                            start=(dc == 0), stop=(dc == 3))
                    ysb = ysbp.tile([128, 512], bf16)
                    nc.scalar.copy(ysb[:], py[:])
                    nc.sync.dma_start(
                        y[(g * 4 + t) * 128:(g * 4 + t + 1) * 128,
                          ec * 512:(ec + 1) * 512],
                        ysb[:])

        fetch_q(0)
        fetch_q(1)
        prev = None
        for s in range(len(slots)):
            if s + 2 < len(slots):
                fetch_q(s + 2)
            qg = qg_t.pop(s)
            pes = [[None, None] for _ in range(4)]
            ds = (dtiles[(s % 2) * 2], dtiles[(s % 2) * 2 + 1])
            stage_a_part(s, (0, 1), qg, pes, ds)
            if prev is not None:
                stage_b(*prev)
                pg, php = slots[prev[0]]
                if php == 3:
                    out_proj(pg)
            stage_a_part(s, (2, 3), qg, pes, ds)
            prev = (s, pes, ds)
        stage_b(*prev)
        out_proj(slots[prev[0]][0])

    with tile.TileContext(nc) as tc:
        with (
            tc.tile_pool(name="const", bufs=1) as constp,
            tc.tile_pool(name="proj_sb", bufs=1) as projp,
            tc.tile_pool(name="hout", bufs=1) as houtp,
            tc.tile_pool(name="dram", bufs=1, space="DRAM") as dramp,
        ):
            ident = constp.tile([128, 128], bf16)
            make_identity(nc, ident[:])
            st["ident"] = ident
            st["projp"] = projp
            st["qT_dram"] = dramp.tile([HGD, N], f32r, name="qT_dram")
            st["hout_sb"] = [houtp.tile([128, N], bf16, name=f"houts{i}")
                             for i in range(4)]

            with (
                tc.tile_pool(name="w1", bufs=1) as w1p,
                tc.tile_pool(name="xg", bufs=2) as xgp,
                tc.tile_pool(name="ef", bufs=2) as efp,
                tc.tile_pool(name="kv", bufs=3) as kvp,
                tc.tile_pool(name="qsb", bufs=2) as qsbp,
                tc.tile_pool(name="ps_q", bufs=2, space="PSUM") as psq,
                tc.tile_pool(name="ps_kv", bufs=1, space="PSUM") as pskv,
                tc.tile_pool(name="ps_proj", bufs=1, space="PSUM") as pspr,
            ):
                st.update(w1p=w1p, xgp=xgp, efp=efp, kvp=kvp, qsbp=qsbp,
                          psq=psq, pskv=pskv, pspr=pspr)
                phase1(tc)

            with tc.tile_pool(name="ps_vp", bufs=1, space="PSUM") as psvp:
                st["psvp"] = psvp
                phase15(tc)

            with tc.tile_pool(name="w3", bufs=1) as w3p:
                wout_s = w3p.tile([128, 4 * DIM], bf16)
                for dc in range(4):
                    nc.sync.dma_start(
                        wout_s[:, dc * DIM:(dc + 1) * DIM],
                        woutT[dc * 128:(dc + 1) * 128, :])
                st["wout_s"] = wout_s
                with (
                    tc.tile_pool(name="qg", bufs=4) as qgp,
                    tc.tile_pool(name="pexp", bufs=16) as pexpp,
                    tc.tile_pool(name="at", bufs=4) as atp,
                    tc.tile_pool(name="vec", bufs=64) as vecp,
                    tc.tile_pool(name="ysb", bufs=3) as ysbp,
                    tc.tile_pool(name="dt", bufs=1) as dtp,
                    tc.tile_pool(name="ps_s", bufs=3, space="PSUM") as pss,
                    tc.tile_pool(name="ps_b", bufs=1, space="PSUM") as psb,
                ):
                    dtiles = [dtp.tile([128, 512], bf16, name=f"dt{i}")
                              for i in range(4)]
                    st.update(qgp=qgp, pexpp=pexpp, atp=atp, vecp=vecp,
                              ysbp=ysbp, pss=pss, psb=psb, dtiles=dtiles)
                    phase2(tc)

    nc.compile()
    return nc


def _prep_inputs(x, Wqkv, E, F, Wout):
    """Build the 8 per-core input dicts (host-side slicing/transposes)."""
    import ml_dtypes
    bf16 = ml_dtypes.bfloat16
    ins = []
    per_hg = {}
    for hg in range(2):
        rr = hg * HGD
        # fold the 1/sqrt(HD) score scale into Wq so scores come out
        # pre-scaled and the softmax bias is just -rowmax
        wqT = np.ascontiguousarray(Wqkv[rr:rr + HGD, :].T) * SCALE
        wk = Wqkv[DIM + rr: DIM + rr + HGD, :]
        wv = Wqkv[2 * DIM + rr: 2 * DIM + rr + HGD, :]
        wkvT = np.ascontiguousarray(np.concatenate([wk.T, wv.T], axis=1))
        eT = np.ascontiguousarray(
            E[hg * HG:(hg + 1) * HG].transpose(2, 0, 1).reshape(N, HG * K))
        fT = np.ascontiguousarray(
            F[hg * HG:(hg + 1) * HG].transpose(2, 0, 1).reshape(N, HG * K)
        ).astype(bf16)
        woutT = np.ascontiguousarray(Wout[:, rr:rr + HGD].T).astype(bf16)
        per_hg[hg] = (wqT, wkvT, eT, fT, woutT)
    xTs = [np.ascontiguousarray(x[b].T) for b in range(B)]
    for c in range(NCORES):
        b, hg = c // 2, c % 2
        wqT, wkvT, eT, fT, woutT = per_hg[hg]
        ins.append({"xT": xTs[b], "wqT": wqT, "wkvT": wkvT,
                    "eT": eT, "fT": fT, "woutT": woutT})
    return ins


def kernel(x, Wqkv, E, F, Wout, bout):
    from concourse.bass_utils import run_bass_kernel_spmd

    x = np.asarray(x, dtype=np.float32)
    Wqkv = np.asarray(Wqkv, dtype=np.float32)
    E = np.asarray(E, dtype=np.float32)
    F = np.asarray(F, dtype=np.float32)
    Wout = np.asarray(Wout, dtype=np.float32)
    bout = np.asarray(bout, dtype=np.float32)

    if "nc" not in _cache:
        _cache["nc"] = _build()
    nc = _cache["nc"]

    in_maps = _prep_inputs(x, Wqkv, E, F, Wout)
    res = run_bass_kernel_spmd(nc, in_maps, core_ids=list(range(NCORES)))
    out = np.empty((B, N, DIM), dtype=np.float32)
    for b in range(B):
        out[b] = (res.results[2 * b]["y"].astype(np.float32)
                  + res.results[2 * b + 1]["y"].astype(np.float32) + bout)
    return out
